# revision 7
# baseline (speedup 1.0000x reference)
"""Trainium2 Bass kernel for nn_BERTTrainer_4947802325049 (segment_reduce).

Computes: ragged span max-pooling over sentence embeddings for (source,
target) phrase pairs + CosineEmbeddingLoss(mean).

Strategy (8 NeuronCores, data-parallel over sentences):
  - core c owns sentences [8c, 8c+8); span pairs are routed to the core that
    owns their sentence (sid//8), in groups of 2 sentences.
  - per 2-sentence group, the core builds sliding-window max tables
    W1/W2/W3 (window 2/4/8) in bf16 next to the bf16 copy of the rows, all
    in one SBUF tile [128 partitions, 32 slots, 1024]; row r lives at
    (partition r//8, slot (level*8 + r%8)).  Window construction is nearly
    all free-axis DVE work; slot-boundary partners come via tiny
    SBUF->SBUF DMAs.
  - any span [a, a+l) is then max(T[a], T[b]) for exactly two rows of one
    table level (host-computed indices), so a single SBUF-source
    transposing dma_gather per group fetches 4*SG rows (s-row1|s-row2|
    t-row1|t-row2) into [128, 8, 4*SG] (D on partitions, spans on free).
  - DVE: two in-place 2-row maxes + s*t product; ACT: s^2, t^2;
    PE: ones-matmul partition-reduction into PSUM [1, SG] per product.
  - host: cos / margin loss / mean over the 8192 pairs.

HBM traffic per core ~= 16 MiB (the embedding shard, read once) + ~12 MiB
SBUF-side gather — vs ~128 MiB for the naive 16-row HBM gather.
"""

import numpy as np

# ---- problem constants (hardcoded per harness contract)
B, S, D, N = 64, 512, 1024, 8192
L_MAX, MARGIN, EPS = 16, 0.4, 1e-8
N_CORES = 8
SENT_PER_CORE = B // N_CORES            # 8
SENT_PER_GROUP = 2
GROUPS = SENT_PER_CORE // SENT_PER_GROUP  # 4
GROUP_ROWS = SENT_PER_GROUP * S         # 1024
SLOTS = GROUP_ROWS // 128               # 8 rows per partition per level
SG = 384                                # padded pairs per (core, group)
NIDX = 4 * SG                           # gather indices per group
NSEG = 4                                # s-row1 | s-row2 | t-row1 | t-row2
IDX_COLS = SG // 16                     # 24 (idx cols per segment call)

_PROGRAM_CACHE = {}


# --------------------------------------------------------------------------
# host-side index prep
# --------------------------------------------------------------------------

def _span_level_rows(pos, ell):
    """Vectorized: span [pos, pos+ell) -> (level, r1, r2), rows rel to
    sentence. max(W[level][r1], W[level][r2]) == max over the span."""
    L = np.select([ell == 1, ell <= 4, ell <= 8], [0, 1, 2], default=3)
    r2off = np.select(
        [ell <= 2, ell <= 4, ell <= 8], [0, ell - 2, ell - 4], default=ell - 8
    )
    return L, pos, pos + r2off


def _encode(L, rloc):
    """(level, row within group) -> gather idx value: partition = rloc//8,
    slot = L*8 + rloc%8, v = slot*128 + partition."""
    return (L * 8 + rloc % 8) * 128 + rloc // 8


def _prep(sid, s_start, s_len, t_start, t_len):
    """Returns (idx16 [N_CORES, GROUPS, NSEG, 128, IDX_COLS] int16,
    pair_map [N_CORES, GROUPS, SG] int64 original pair index or -1)."""
    ell_s = s_len + 1
    ell_t = t_len + 1
    Ls, s_r1, s_r2 = _span_level_rows(s_start, ell_s)
    Lt, t_r1, t_r2 = _span_level_rows(t_start, ell_t)
    base = (sid % SENT_PER_GROUP) * S
    segs = (
        _encode(Ls, base + s_r1),
        _encode(Ls, base + s_r2),
        _encode(Lt, base + t_r1),
        _encode(Lt, base + t_r2),
    )

    core = sid // SENT_PER_CORE
    grp = (sid % SENT_PER_CORE) // SENT_PER_GROUP

    idx16 = np.zeros((N_CORES, GROUPS, NSEG, 128, IDX_COLS), dtype=np.int16)
    pair_map = np.full((N_CORES, GROUPS, SG), -1, dtype=np.int64)
    pos_i = np.arange(SG)
    prow = (pos_i % 16).astype(np.int64)
    pcol = pos_i // 16
    for c in range(N_CORES):
        for g in range(GROUPS):
            j = np.nonzero((core == c) & (grp == g))[0]
            n = len(j)
            if n > SG:
                raise ValueError(f"group overflow: {n} > {SG}")
            pair_map[c, g, :n] = j
            for q in range(NSEG):
                vals = np.zeros(SG, dtype=np.int16)
                vals[0:n] = segs[q][j]
                block = np.zeros((16, IDX_COLS), dtype=np.int16)
                block[prow, pcol] = vals
                idx16[c, g, q] = np.tile(block, (8, 1))
    return idx16, pair_map


# --------------------------------------------------------------------------
# device program
# --------------------------------------------------------------------------

def _build_program():
    import concourse.bacc as bacc
    import concourse.mybir as mybir
    import concourse.tile as tile

    nc = bacc.Bacc("TRN2", target_bir_lowering=False, debug=False)
    bf16 = mybir.dt.bfloat16
    f32 = mybir.dt.float32
    MAX = mybir.AluOpType.max
    MULT = mybir.AluOpType.mult

    emb = nc.dram_tensor("emb", [SENT_PER_CORE * S, D], f32, kind="ExternalInput")
    gidx = nc.dram_tensor(
        "gidx", [GROUPS, NSEG, 128, IDX_COLS], mybir.dt.int16, kind="ExternalInput"
    )
    res = nc.dram_tensor("res", [GROUPS, 3 * SG], f32, kind="ExternalOutput")

    with tile.TileContext(nc) as tc:
        with (
            tc.tile_pool(name="tab", bufs=2) as tabp,
            tc.tile_pool(name="dst", bufs=1) as dstp,
            tc.tile_pool(name="nxt", bufs=2) as nxtp,
            tc.tile_pool(name="sml", bufs=1) as smlp,
            tc.tile_pool(name="fin", bufs=2) as finp,
            tc.tile_pool(name="ps", bufs=2, space="PSUM") as psp,
        ):
            ones = smlp.tile([128, 1], bf16)
            nc.gpsimd.memset(ones[:], 1.0)
            idx_sb = smlp.tile([128, GROUPS, NSEG, IDX_COLS], mybir.dt.int16)
            nc.sync.dma_start(
                out=idx_sb[:], in_=gidx[:].rearrange("g q p c -> p g q c")
            )

            for g in range(GROUPS):
                # combined table tile: slots [0:8) E, [8:16) W1, [16:24) W2,
                # [24:32) W3; row r of the group -> (partition r//8, slot r%8)
                comb = tabp.tile([128, 4 * SLOTS, D], bf16, tag="comb")

                # E (bf16 cast during the HBM load)
                nc.gpsimd.dma_start(
                    out=comb[:, 0:SLOTS, :],
                    in_=emb[g * GROUP_ROWS : (g + 1) * GROUP_ROWS, :].rearrange(
                        "(p s) d -> p s d", p=128, s=SLOTS
                    ),
                )

                # W_k[r] = max(W_{k-1}[r], W_{k-1}[r+delta])
                for lvl, delta in ((1, 1), (2, 2), (3, 4)):
                    src = (lvl - 1) * SLOTS
                    dst = lvl * SLOTS
                    k = SLOTS - delta
                    nc.vector.tensor_tensor(
                        out=comb[:, dst : dst + k, :],
                        in0=comb[:, src : src + k, :],
                        in1=comb[:, src + delta : src + SLOTS, :],
                        op=MAX,
                    )
                    # boundary slots: partner rows live in the next partition
                    nxt = nxtp.tile([128, 4, D], bf16, tag="nxt")
                    nc.sync.dma_start(
                        out=nxt[0:127, 0:delta, :],
                        in_=comb[1:128, src : src + delta, :],
                    )
                    nc.vector.tensor_tensor(
                        out=comb[0:127, dst + k : dst + SLOTS, :],
                        in0=comb[0:127, src + k : src + SLOTS, :],
                        in1=nxt[0:127, 0:delta, :],
                        op=MAX,
                    )
                    # partition 127 tail rows: clipped (values never gathered,
                    # copied to stay finite)
                    nc.sync.dma_start(
                        out=comb[127:128, dst + k : dst + SLOTS, :],
                        in_=comb[127:128, src + k : src + SLOTS, :],
                    )

                # gather 4*SG rows (one call per segment; a single call of
                # 4*SG x 2KiB overflows the SWDGE descriptor ring),
                # transposing to (D on partitions)
                seg = []
                for q in range(NSEG):
                    dq = dstp.tile([128, SLOTS, SG], bf16, tag=f"dest{q}")
                    nc.gpsimd.dma_gather(
                        out_ap=dq[:],
                        in_ap=comb[:].rearrange("p s d -> p (s d)"),
                        idxs_ap=idx_sb[:, g, q, :],
                        num_idxs=SG,
                        num_idxs_reg=SG,
                        elem_size=D,
                        transpose=True,
                        sbuf_tokens_per_rank=128,
                        sbuf_free_dim_per_rank=2 * D,
                        sbuf_free_dim_pad_per_rank=0,
                        sbuf_byte_offset=0,
                    )
                    seg.append(dq)

                # span vectors: in-place 2-row maxes
                sv, s2, tv, t2 = (x[:] for x in seg)
                nc.vector.tensor_tensor(out=sv, in0=sv, in1=s2, op=MAX)
                nc.vector.tensor_tensor(out=tv, in0=tv, in1=t2, op=MAX)
                # products: st into the old s-row2 slot, ss into old t-row2
                nc.vector.tensor_tensor(out=s2, in0=sv, in1=tv, op=MULT)
                nc.scalar.activation(
                    out=t2, in_=sv, func=mybir.ActivationFunctionType.Square
                )

                acc_st = psp.tile([1, SG], f32, tag="acc_st", space="PSUM")
                acc_ss = psp.tile([1, SG], f32, tag="acc_ss", space="PSUM")
                acc_tt = psp.tile([1, SG], f32, tag="acc_tt", space="PSUM")
                for c in range(SLOTS):
                    nc.tensor.matmul(
                        out=acc_st[:],
                        lhsT=ones[:],
                        rhs=seg[1][:, c, :],
                        start=(c == 0),
                        stop=(c == SLOTS - 1),
                    )
                for c in range(SLOTS):
                    nc.tensor.matmul(
                        out=acc_ss[:],
                        lhsT=ones[:],
                        rhs=seg[3][:, c, :],
                        start=(c == 0),
                        stop=(c == SLOTS - 1),
                    )
                # tt reuses the st-product slot once its matmuls are done
                nc.scalar.activation(
                    out=s2, in_=tv, func=mybir.ActivationFunctionType.Square
                )
                for c in range(SLOTS):
                    nc.tensor.matmul(
                        out=acc_tt[:],
                        lhsT=ones[:],
                        rhs=seg[1][:, c, :],
                        start=(c == 0),
                        stop=(c == SLOTS - 1),
                    )

                fin = finp.tile([1, 3 * SG], f32, tag="fin")
                nc.scalar.copy(out=fin[0:1, 0:SG], in_=acc_st[:])
                nc.scalar.copy(out=fin[0:1, SG : 2 * SG], in_=acc_ss[:])
                nc.scalar.copy(out=fin[0:1, 2 * SG : 3 * SG], in_=acc_tt[:])
                nc.sync.dma_start(out=res[g : g + 1, :], in_=fin[:])

    nc.compile()
    return nc


def _get_program():
    if "nc" not in _PROGRAM_CACHE:
        _PROGRAM_CACHE["nc"] = _build_program()
    return _PROGRAM_CACHE["nc"]


def _run_on_device(in_maps, trace=False, **kw):
    from concourse import bass_utils

    nc = _get_program()
    return bass_utils.run_bass_kernel_spmd(
        nc, in_maps, core_ids=list(range(N_CORES)), trace=trace, **kw
    )


def _make_in_maps(sent_emb, idx16):
    in_maps = []
    for c in range(N_CORES):
        emb_c = np.ascontiguousarray(
            sent_emb[c * SENT_PER_CORE : (c + 1) * SENT_PER_CORE]
        ).reshape(SENT_PER_CORE * S, D)
        in_maps.append({"emb": emb_c, "gidx": np.ascontiguousarray(idx16[c])})
    return in_maps


def _finalize(results, pair_map, labels):
    dots = np.zeros(N)
    sss = np.zeros(N)
    tts = np.zeros(N)
    for c in range(N_CORES):
        r = np.asarray(results[c]["res"], dtype=np.float64).reshape(GROUPS, 3, SG)
        for g in range(GROUPS):
            m = pair_map[c, g]
            valid = m >= 0
            dots[m[valid]] = r[g, 0, valid]
            sss[m[valid]] = r[g, 1, valid]
            tts[m[valid]] = r[g, 2, valid]
    cos = dots / (np.sqrt(sss) * np.sqrt(tts) + EPS)
    y = 2.0 * labels.astype(np.float64) - 1.0
    per_pair = np.where(y > 0, 1.0 - cos, np.maximum(0.0, cos - MARGIN))
    return np.float32(per_pair.mean())


def kernel(sent_emb, sid, s_start, s_len, t_start, t_len, labels):
    sent_emb = np.asarray(sent_emb, dtype=np.float32)
    sid = np.asarray(sid)
    idx16, pair_map = _prep(
        np.asarray(sid, dtype=np.int64),
        np.asarray(s_start, dtype=np.int64),
        np.asarray(s_len, dtype=np.int64),
        np.asarray(t_start, dtype=np.int64),
        np.asarray(t_len, dtype=np.int64),
    )
    in_maps = _make_in_maps(sent_emb, idx16)
    out = _run_on_device(in_maps)
    return _finalize(out.results, pair_map, np.asarray(labels))


# revision 8
# speedup vs baseline: 1.1059x; 1.1059x over previous
"""Trainium2 Bass kernel for nn_BERTTrainer_4947802325049 (segment_reduce).

Computes: ragged span max-pooling over sentence embeddings for (source,
target) phrase pairs + CosineEmbeddingLoss(mean).

Strategy (8 NeuronCores, data-parallel over sentences):
  - core c owns sentences [8c, 8c+8); span pairs are routed to the core that
    owns their sentence (sid//8), in groups of 2 sentences.
  - per 2-sentence group, the core builds sliding-window max tables
    W1/W2/W3 (window 2/4/8) in bf16 next to the bf16 copy of the rows, all
    in one SBUF tile [128 partitions, 32 slots, 1024]; row r lives at
    (partition r//8, slot (level*8 + r%8)).  Window construction is nearly
    all free-axis DVE work; slot-boundary partners come via tiny
    SBUF->SBUF DMAs.
  - any span [a, a+l) is then max(T[a], T[b]) for exactly two rows of one
    table level (host-computed indices), so a single SBUF-source
    transposing dma_gather per group fetches 4*SG rows (s-row1|s-row2|
    t-row1|t-row2) into [128, 8, 4*SG] (D on partitions, spans on free).
  - DVE: two in-place 2-row maxes + s*t product; ACT: s^2, t^2;
    PE: ones-matmul partition-reduction into PSUM [1, SG] per product.
  - host: cos / margin loss / mean over the 8192 pairs.

HBM traffic per core ~= 16 MiB (the embedding shard, read once) + ~12 MiB
SBUF-side gather — vs ~128 MiB for the naive 16-row HBM gather.
"""

import numpy as np

# ---- problem constants (hardcoded per harness contract)
B, S, D, N = 64, 512, 1024, 8192
L_MAX, MARGIN, EPS = 16, 0.4, 1e-8
N_CORES = 8
SENT_PER_CORE = B // N_CORES            # 8
SENT_PER_GROUP = 2
GROUPS = SENT_PER_CORE // SENT_PER_GROUP  # 4
GROUP_ROWS = SENT_PER_GROUP * S         # 1024
SLOTS = GROUP_ROWS // 128               # 8 rows per partition per level
SG = 384                                # padded pairs per (core, group)
NIDX = 4 * SG                           # gather indices per group
NSEG = 4                                # s-row1 | s-row2 | t-row1 | t-row2
IDX_COLS = SG // 16                     # 24 (idx cols per segment call)

_PROGRAM_CACHE = {}


# --------------------------------------------------------------------------
# host-side index prep
# --------------------------------------------------------------------------

def _span_level_rows(pos, ell):
    """Vectorized: span [pos, pos+ell) -> (level, r1, r2), rows rel to
    sentence. max(W[level][r1], W[level][r2]) == max over the span."""
    L = np.select([ell == 1, ell <= 4, ell <= 8], [0, 1, 2], default=3)
    r2off = np.select(
        [ell <= 2, ell <= 4, ell <= 8], [0, ell - 2, ell - 4], default=ell - 8
    )
    return L, pos, pos + r2off


def _encode(L, rloc):
    """(level, row within group) -> gather idx value: partition = rloc//8,
    slot = L*8 + rloc%8, v = slot*128 + partition."""
    return (L * 8 + rloc % 8) * 128 + rloc // 8


def _prep(sid, s_start, s_len, t_start, t_len):
    """Returns (idx16 [N_CORES, GROUPS, NSEG, 128, IDX_COLS] int16,
    pair_map [N_CORES, GROUPS, SG] int64 original pair index or -1)."""
    ell_s = s_len + 1
    ell_t = t_len + 1
    Ls, s_r1, s_r2 = _span_level_rows(s_start, ell_s)
    Lt, t_r1, t_r2 = _span_level_rows(t_start, ell_t)
    base = (sid % SENT_PER_GROUP) * S
    segs = (
        _encode(Ls, base + s_r1),
        _encode(Ls, base + s_r2),
        _encode(Lt, base + t_r1),
        _encode(Lt, base + t_r2),
    )

    core = sid // SENT_PER_CORE
    grp = (sid % SENT_PER_CORE) // SENT_PER_GROUP

    idx16 = np.zeros((N_CORES, GROUPS, NSEG, 128, IDX_COLS), dtype=np.int16)
    pair_map = np.full((N_CORES, GROUPS, SG), -1, dtype=np.int64)
    pos_i = np.arange(SG)
    prow = (pos_i % 16).astype(np.int64)
    pcol = pos_i // 16
    for c in range(N_CORES):
        for g in range(GROUPS):
            j = np.nonzero((core == c) & (grp == g))[0]
            n = len(j)
            if n > SG:
                raise ValueError(f"group overflow: {n} > {SG}")
            pair_map[c, g, :n] = j
            for q in range(NSEG):
                vals = np.zeros(SG, dtype=np.int16)
                vals[0:n] = segs[q][j]
                block = np.zeros((16, IDX_COLS), dtype=np.int16)
                block[prow, pcol] = vals
                idx16[c, g, q] = np.tile(block, (8, 1))
    return idx16, pair_map


# --------------------------------------------------------------------------
# device program
# --------------------------------------------------------------------------

def _build_program():
    import concourse.bacc as bacc
    import concourse.mybir as mybir
    import concourse.tile as tile

    nc = bacc.Bacc("TRN2", target_bir_lowering=False, debug=False)
    bf16 = mybir.dt.bfloat16
    f32 = mybir.dt.float32
    MAX = mybir.AluOpType.max
    MULT = mybir.AluOpType.mult

    emb = nc.dram_tensor("emb", [SENT_PER_CORE * S, D], f32, kind="ExternalInput")
    gidx = nc.dram_tensor(
        "gidx", [GROUPS, NSEG, 128, IDX_COLS], mybir.dt.int16, kind="ExternalInput"
    )
    res = nc.dram_tensor("res", [GROUPS, 3 * SG], f32, kind="ExternalOutput")

    with tile.TileContext(nc) as tc:
        with (
            tc.tile_pool(name="tab", bufs=2) as tabp,
            tc.tile_pool(name="dst", bufs=1) as dstp,
            tc.tile_pool(name="nxt", bufs=2) as nxtp,
            tc.tile_pool(name="sml", bufs=1) as smlp,
            tc.tile_pool(name="fin", bufs=2) as finp,
            tc.tile_pool(name="ps", bufs=2, space="PSUM") as psp,
        ):
            ones = smlp.tile([128, 1], bf16)
            nc.gpsimd.memset(ones[:], 1.0)
            idx_sb = smlp.tile([128, GROUPS, NSEG, IDX_COLS], mybir.dt.int16)
            nc.sync.dma_start(
                out=idx_sb[:], in_=gidx[:].rearrange("g q p c -> p g q c")
            )

            for g in range(GROUPS):
                # combined table tile: slots [0:8) E, [8:16) W1, [16:24) W2,
                # [24:32) W3; row r of the group -> (partition r//8, slot r%8)
                comb = tabp.tile([128, 4 * SLOTS, D], bf16, tag="comb")

                # E (bf16 cast during the HBM load)
                nc.gpsimd.dma_start(
                    out=comb[:, 0:SLOTS, :],
                    in_=emb[g * GROUP_ROWS : (g + 1) * GROUP_ROWS, :].rearrange(
                        "(p s) d -> p s d", p=128, s=SLOTS
                    ),
                )

                # W_k[r] = max(W_{k-1}[r], W_{k-1}[r+delta])
                for lvl, delta in ((1, 1), (2, 2), (3, 4)):
                    src = (lvl - 1) * SLOTS
                    dst = lvl * SLOTS
                    k = SLOTS - delta
                    nc.vector.tensor_tensor(
                        out=comb[:, dst : dst + k, :],
                        in0=comb[:, src : src + k, :],
                        in1=comb[:, src + delta : src + SLOTS, :],
                        op=MAX,
                    )
                    # boundary slots: partner rows live in the next partition
                    nxt = nxtp.tile([128, 4, D], bf16, tag="nxt")
                    nc.gpsimd.dma_start(
                        out=nxt[0:127, 0:delta, :],
                        in_=comb[1:128, src : src + delta, :],
                    )
                    nc.vector.tensor_tensor(
                        out=comb[0:127, dst + k : dst + SLOTS, :],
                        in0=comb[0:127, src + k : src + SLOTS, :],
                        in1=nxt[0:127, 0:delta, :],
                        op=MAX,
                    )
                    # partition 127 tail rows: clipped (values never gathered,
                    # copied to stay finite)
                    nc.gpsimd.dma_start(
                        out=comb[127:128, dst + k : dst + SLOTS, :],
                        in_=comb[127:128, src + k : src + SLOTS, :],
                    )

                # gather 4*SG rows (one call per segment; a single call of
                # 4*SG x 2KiB overflows the SWDGE descriptor ring),
                # transposing to (D on partitions)
                seg = []
                for q in range(NSEG):
                    dq = dstp.tile([128, SLOTS, SG], bf16, tag=f"dest{q}")
                    nc.gpsimd.dma_gather(
                        out_ap=dq[:],
                        in_ap=comb[:].rearrange("p s d -> p (s d)"),
                        idxs_ap=idx_sb[:, g, q, :],
                        num_idxs=SG,
                        num_idxs_reg=SG,
                        elem_size=D,
                        transpose=True,
                        sbuf_tokens_per_rank=128,
                        sbuf_free_dim_per_rank=2 * D,
                        sbuf_free_dim_pad_per_rank=0,
                        sbuf_byte_offset=0,
                    )
                    seg.append(dq)

                # span vectors: in-place 2-row maxes
                sv, s2, tv, t2 = (x[:] for x in seg)
                nc.vector.tensor_tensor(out=sv, in0=sv, in1=s2, op=MAX)
                nc.vector.tensor_tensor(out=tv, in0=tv, in1=t2, op=MAX)
                # products: st into the old s-row2 slot, ss into old t-row2
                nc.vector.tensor_tensor(out=s2, in0=sv, in1=tv, op=MULT)
                nc.scalar.activation(
                    out=t2, in_=sv, func=mybir.ActivationFunctionType.Square
                )

                acc_st = psp.tile([1, SG], f32, tag="acc_st", space="PSUM")
                acc_ss = psp.tile([1, SG], f32, tag="acc_ss", space="PSUM")
                acc_tt = psp.tile([1, SG], f32, tag="acc_tt", space="PSUM")
                for c in range(SLOTS):
                    nc.tensor.matmul(
                        out=acc_st[:],
                        lhsT=ones[:],
                        rhs=seg[1][:, c, :],
                        start=(c == 0),
                        stop=(c == SLOTS - 1),
                    )
                for c in range(SLOTS):
                    nc.tensor.matmul(
                        out=acc_ss[:],
                        lhsT=ones[:],
                        rhs=seg[3][:, c, :],
                        start=(c == 0),
                        stop=(c == SLOTS - 1),
                    )
                # tt reuses the st-product slot once its matmuls are done
                nc.scalar.activation(
                    out=s2, in_=tv, func=mybir.ActivationFunctionType.Square
                )
                for c in range(SLOTS):
                    nc.tensor.matmul(
                        out=acc_tt[:],
                        lhsT=ones[:],
                        rhs=seg[1][:, c, :],
                        start=(c == 0),
                        stop=(c == SLOTS - 1),
                    )

                fin = finp.tile([1, 3 * SG], f32, tag="fin")
                nc.scalar.copy(out=fin[0:1, 0:SG], in_=acc_st[:])
                nc.scalar.copy(out=fin[0:1, SG : 2 * SG], in_=acc_ss[:])
                nc.scalar.copy(out=fin[0:1, 2 * SG : 3 * SG], in_=acc_tt[:])
                nc.gpsimd.dma_start(out=res[g : g + 1, :], in_=fin[:])

    nc.compile()
    return nc


def _get_program():
    if "nc" not in _PROGRAM_CACHE:
        _PROGRAM_CACHE["nc"] = _build_program()
    return _PROGRAM_CACHE["nc"]


def _run_on_device(in_maps, trace=False, **kw):
    from concourse import bass_utils

    nc = _get_program()
    return bass_utils.run_bass_kernel_spmd(
        nc, in_maps, core_ids=list(range(N_CORES)), trace=trace, **kw
    )


def _make_in_maps(sent_emb, idx16):
    in_maps = []
    for c in range(N_CORES):
        emb_c = np.ascontiguousarray(
            sent_emb[c * SENT_PER_CORE : (c + 1) * SENT_PER_CORE]
        ).reshape(SENT_PER_CORE * S, D)
        in_maps.append({"emb": emb_c, "gidx": np.ascontiguousarray(idx16[c])})
    return in_maps


def _finalize(results, pair_map, labels):
    dots = np.zeros(N)
    sss = np.zeros(N)
    tts = np.zeros(N)
    for c in range(N_CORES):
        r = np.asarray(results[c]["res"], dtype=np.float64).reshape(GROUPS, 3, SG)
        for g in range(GROUPS):
            m = pair_map[c, g]
            valid = m >= 0
            dots[m[valid]] = r[g, 0, valid]
            sss[m[valid]] = r[g, 1, valid]
            tts[m[valid]] = r[g, 2, valid]
    cos = dots / (np.sqrt(sss) * np.sqrt(tts) + EPS)
    y = 2.0 * labels.astype(np.float64) - 1.0
    per_pair = np.where(y > 0, 1.0 - cos, np.maximum(0.0, cos - MARGIN))
    return np.float32(per_pair.mean())


def kernel(sent_emb, sid, s_start, s_len, t_start, t_len, labels):
    sent_emb = np.asarray(sent_emb, dtype=np.float32)
    sid = np.asarray(sid)
    idx16, pair_map = _prep(
        np.asarray(sid, dtype=np.int64),
        np.asarray(s_start, dtype=np.int64),
        np.asarray(s_len, dtype=np.int64),
        np.asarray(t_start, dtype=np.int64),
        np.asarray(t_len, dtype=np.int64),
    )
    in_maps = _make_in_maps(sent_emb, idx16)
    out = _run_on_device(in_maps)
    return _finalize(out.results, pair_map, np.asarray(labels))
